# revision 9
# baseline (speedup 1.0000x reference)
"""Trainium2 Bass kernel for DWPEDecomposition.

Problem: x [128, 65536] f32.  For each batch row: full 8-level Haar (db1)
wavelet-packet tree -> [256 nodes, 256 coeffs] in frequency order, per-node
Shannon entropy of the normalized squared coefficients, and a keep mask
(entropy > 0.1) that zeroes pruned nodes' coefficients.

Key insight: the 8-level Haar packet cascade on a 65536-sample row is exactly
a 256x256 Walsh-Hadamard-style matrix W' (entries +-(1/sqrt2)^8, with the
frequency-order node permutation folded in) applied independently to each of
the 256 contiguous 256-sample blocks:

    coeffs[b, n, k] = sum_j W'[n, j] * x[b, 256*k + j]

which maps onto the TensorEngine as (PE-transpose + matmul) per tile.
Entropy per node n uses the identity
    -sum_k q ln q  =  (ln(Se) * S - G) / Se,   q = sq/Se,
    S = sum_k c^2,  Se = S + 1e-8,  G = sum_k sq*ln(sq)
computed with ACT (Square/Ln), DVE (segmented reduces, small vector math) and
GPSIMD (sq*lq product).

Sharding: pure data parallel, 16 batch rows per core across 8 NeuronCores.
"""
import sys

sys.path.insert(0, "/opt/trn_rl_repo")

import numpy as np

import concourse.bass as bass
import concourse.tile as tile
from concourse import bacc, mybir
from concourse import bass_utils

F32 = mybir.dt.float32
F32R = mybir.dt.float32r
ALU = mybir.AluOpType
ACTFN = mybir.ActivationFunctionType

N_CORES = 8
B = 128                  # total batch rows
T = 65536                # samples per row
ROWS = B // N_CORES      # rows per core (16)
LEVEL = 8
NODES = 1 << LEVEL       # 256
BLK = T // NODES         # 256 samples per block
THRESHOLD = 0.1
RPB = 2                  # rows per batch (inner tile loop)
NBATCH = ROWS // RPB     # 8 batches per core

_INV = np.float32(0.7071067811865476)
_g = np.arange(NODES)
_FREQ_PERM = np.argsort(_g ^ (_g >> 1))


def _build_w():
    """W'[n, j]: response of freq-ordered node n to impulse at in-block pos j.

    Built by running the (numpy, float32) cascade on the identity, which
    reproduces the reference arithmetic bit-for-bit per entry.
    """
    c = np.eye(BLK, dtype=np.float32)[:, None, :]
    for _ in range(LEVEL):
        ev = c[..., 0::2]
        od = c[..., 1::2]
        a = (ev + od) * _INV
        d = (ev - od) * _INV
        c = np.concatenate([a[:, :, None, :], d[:, :, None, :]], axis=2)
        c = c.reshape(BLK, -1, a.shape[-1])
    w = c[:, _FREQ_PERM, 0].T.copy()  # [n, j]
    return w


W = _build_w()
# lhsT chunks: packed [128, 512] with wt[j', jc*256 + nh*128 + m] = W[nh*128+m, jc*128+j']
WT_PACKED = np.hstack([W.T[0:128, :], W.T[128:256, :]]).astype(np.float32).copy()
IDENT = np.eye(128, dtype=np.float32)

_MODULE_CACHE = None


def _build_module():
    nc = bacc.Bacc("TRN2", target_bir_lowering=False, debug=False,
                   enable_asserts=False, num_devices=N_CORES)
    x_d = nc.dram_tensor("x", [ROWS, T], F32, kind="ExternalInput").ap()
    wt_d = nc.dram_tensor("wt", [128, 512], F32, kind="ExternalInput").ap()
    id_d = nc.dram_tensor("ident", [128, 128], F32, kind="ExternalInput").ap()
    out_d = nc.dram_tensor("out", [ROWS, T], F32, kind="ExternalOutput").ap()
    ent_d = nc.dram_tensor("ent", [ROWS, NODES], F32, kind="ExternalOutput").ap()

    FREE = RPB * 512     # free-dim elems per batch tile (2 rows x 512)

    with tile.TileContext(nc) as tc:
        with (
            tc.tile_pool(name="const", bufs=1) as const_pool,
            tc.tile_pool(name="xin", bufs=3) as xin_pool,
            tc.tile_pool(name="xt_ps", bufs=2, space="PSUM") as xtps_pool,
            tc.tile_pool(name="c_ps", bufs=2, space="PSUM") as cps_pool,
            tc.tile_pool(name="xt_sb", bufs=2) as xtsb_pool,
            tc.tile_pool(name="sq", bufs=2) as sq_pool,
            tc.tile_pool(name="lq", bufs=2) as lq_pool,
            tc.tile_pool(name="tt", bufs=2) as tt_pool,
            tc.tile_pool(name="outs", bufs=3) as out_pool,
            tc.tile_pool(name="stats", bufs=3) as stat_pool,
        ):
            wt_sb = const_pool.tile([128, 512], F32)
            nc.sync.dma_start(wt_sb[:], wt_d)
            ident = const_pool.tile([128, 128], F32)
            nc.sync.dma_start(ident[:], id_d)
            ent_sb = const_pool.tile([128, 2 * ROWS], F32)
            bias_tiny = const_pool.tile([128, 1], F32)
            nc.gpsimd.memset(bias_tiny[:], 1e-30)

            for bi in range(NBATCH):
                r0 = bi * RPB
                x_sb = xin_pool.tile([128, FREE], F32)
                src = x_d[r0:r0 + RPB].rearrange(
                    "r (cj p j) -> p r cj j", cj=2, p=128, j=BLK)
                nc.sync.dma_start(x_sb[:], src)

                # PE transposes: per row rl, chunk (cj, jc):
                #   xt[:, rl*512 + (jc*2+cj)*128 + k'] , partitions j'
                xt = xtps_pool.tile([128, FREE], F32)
                for rl in range(RPB):
                    for cj in range(2):
                        for jc in range(2):
                            nc.tensor.transpose(
                                xt[:, rl * 512 + (jc * 2 + cj) * 128:
                                   rl * 512 + (jc * 2 + cj + 1) * 128],
                                x_sb[:, rl * 512 + cj * 256 + jc * 128:
                                     rl * 512 + cj * 256 + (jc + 1) * 128],
                                ident[:],
                            )
                xts = xtsb_pool.tile([128, FREE], F32)
                nc.vector.tensor_copy(xts[:], xt[:])

                # matmuls: c[:, rl*512 + nh*256 + k] = coeffs[nh*128+n', k]
                c = cps_pool.tile([128, FREE], F32)
                for rl in range(RPB):
                    for nh in range(2):
                        for jc in range(2):
                            nc.tensor.matmul(
                                c[:, rl * 512 + nh * 256: rl * 512 + (nh + 1) * 256],
                                wt_sb[:, jc * 256 + nh * 128:
                                      jc * 256 + (nh + 1) * 128],
                                xts[:, rl * 512 + jc * 256:
                                    rl * 512 + (jc + 1) * 256],
                                start=(jc == 0), stop=(jc == 1),
                            )

                # entropy pipeline
                sq = sq_pool.tile([128, FREE], F32)
                nc.scalar.square(sq[:], c[:])
                s4 = stat_pool.tile([128, 2 * RPB], F32, tag="s4")
                nc.vector.tensor_reduce(
                    s4[:], sq[:].rearrange("p (g k) -> p g k", k=BLK),
                    axis=mybir.AxisListType.X, op=ALU.add)
                lq = lq_pool.tile([128, FREE], F32)
                nc.scalar.activation(lq[:], sq[:], ACTFN.Ln, bias=bias_tiny[:])
                t = tt_pool.tile([128, FREE], F32)
                nc.gpsimd.tensor_tensor(t[:], sq[:], lq[:], op=ALU.mult)
                g4 = stat_pool.tile([128, 2 * RPB], F32, tag="g4")
                nc.vector.tensor_reduce(
                    g4[:], t[:].rearrange("p (g k) -> p g k", k=BLK),
                    axis=mybir.AxisListType.X, op=ALU.add)

                se = stat_pool.tile([128, 2 * RPB], F32, tag="se")
                nc.vector.tensor_scalar_add(se[:], s4[:], 1e-8)
                sinv = stat_pool.tile([128, 2 * RPB], F32, tag="sinv")
                nc.vector.reciprocal(sinv[:], se[:])
                lns = stat_pool.tile([128, 2 * RPB], F32, tag="lns")
                nc.scalar.activation(lns[:], se[:], ACTFN.Ln)
                u = stat_pool.tile([128, 2 * RPB], F32, tag="u")
                nc.vector.tensor_tensor(u[:], s4[:], lns[:], op=ALU.mult)
                v = stat_pool.tile([128, 2 * RPB], F32, tag="v")
                nc.vector.tensor_tensor(v[:], u[:], g4[:], op=ALU.subtract)
                e = ent_sb[:, 2 * r0: 2 * (r0 + RPB)]
                nc.vector.tensor_tensor(e, v[:], sinv[:], op=ALU.mult)
                m4 = stat_pool.tile([128, 2 * RPB], F32, tag="m4")
                nc.vector.tensor_scalar(m4[:], e, THRESHOLD, None, op0=ALU.is_gt)

                o = out_pool.tile([128, FREE], F32)
                for rl in range(RPB):
                    for nh in range(2):
                        idx = rl * 2 + nh
                        csl = c[:, rl * 512 + nh * 256: rl * 512 + (nh + 1) * 256]
                        osl = o[:, rl * 512 + nh * 256: rl * 512 + (nh + 1) * 256]
                        msl = m4[:, idx:idx + 1]
                        if idx % 2 == 0:
                            nc.vector.tensor_scalar_mul(osl, csl, msl)
                        else:
                            nc.scalar.activation(osl, csl, ACTFN.Copy, scale=msl)
                dst = out_d[r0:r0 + RPB].rearrange(
                    "r (nh p k) -> p r nh k", nh=2, p=128, k=BLK)
                nc.sync.dma_start(dst, o[:])

            # entropy epilogue: [128 n', 32 (r,nh)] -> transpose -> [32, 128] -> DRAM
            entT_ps = xtps_pool.tile([128, 128], F32, tag="xt")
            nc.tensor.transpose(entT_ps[0:2 * ROWS, 0:128], ent_sb[:], ident[:])
            entT = stat_pool.tile([2 * ROWS, 128], F32, tag="entT_sb")
            nc.vector.tensor_copy(entT[:], entT_ps[0:2 * ROWS, 0:128])
            nc.sync.dma_start(
                ent_d.rearrange("r (nh n) -> (r nh) n", nh=2), entT[:])

    nc.compile()
    return nc


def _get_module():
    global _MODULE_CACHE
    if _MODULE_CACHE is None:
        _MODULE_CACHE = _build_module()
    return _MODULE_CACHE


def kernel(x: np.ndarray) -> tuple[np.ndarray, np.ndarray, np.ndarray]:
    x = np.ascontiguousarray(np.asarray(x, dtype=np.float32))
    assert x.shape == (B, T)
    nc = _get_module()
    in_maps = []
    for core in range(N_CORES):
        shard = x[core * ROWS:(core + 1) * ROWS]
        in_maps.append({"x": shard, "wt": WT_PACKED, "ident": IDENT})
    res = bass_utils.run_bass_kernel_spmd(nc, in_maps, core_ids=list(range(N_CORES)))
    global LAST_RESULTS
    LAST_RESULTS = res
    out = np.empty((B, NODES, BLK), dtype=np.float32)
    ent = np.empty((B, NODES), dtype=np.float32)
    for core in range(N_CORES):
        r = res.results[core]
        out[core * ROWS:(core + 1) * ROWS] = r["out"].reshape(ROWS, NODES, BLK)
        ent[core * ROWS:(core + 1) * ROWS] = r["ent"]
    keep = ent > np.float32(THRESHOLD)
    return out, ent, keep


# revision 15
# speedup vs baseline: 1.2711x; 1.2711x over previous
"""Trainium2 Bass kernel for DWPEDecomposition.

Problem: x [128, 65536] f32.  For each batch row: full 8-level Haar (db1)
wavelet-packet tree -> [256 nodes, 256 coeffs] in frequency order, per-node
Shannon entropy of the normalized squared coefficients, and a keep mask
(entropy > 0.1) that zeroes pruned nodes' coefficients.

Key insight: the 8-level Haar packet cascade on a 65536-sample row is exactly
a 256x256 Walsh-Hadamard-style matrix W' (entries +-(1/sqrt2)^8, with the
frequency-order node permutation folded in) applied independently to each of
the 256 contiguous 256-sample blocks:

    coeffs[b, n, k] = sum_j W'[n, j] * x[b, 256*k + j]

which maps onto the TensorEngine as (PE-transpose + matmul) per tile.
Entropy per node n uses the identity
    -sum_k q ln q  =  (ln(Se) * S - G) / Se,   q = sq/Se,
    S = sum_k c^2,  Se = S + 1e-8,  G = sum_k sq*ln(sq)
computed with ACT (Square/Ln), DVE (segmented reduces, small vector math) and
GPSIMD (sq*lq product).

Sharding: pure data parallel, 16 batch rows per core across 8 NeuronCores.
"""
import sys

sys.path.insert(0, "/opt/trn_rl_repo")

import numpy as np

import concourse.bass as bass
import concourse.tile as tile
from concourse import bacc, mybir
from concourse import bass_utils

F32 = mybir.dt.float32
F32R = mybir.dt.float32r
ALU = mybir.AluOpType
ACTFN = mybir.ActivationFunctionType

N_CORES = 8
B = 128                  # total batch rows
T = 65536                # samples per row
ROWS = B // N_CORES      # rows per core (16)
LEVEL = 8
NODES = 1 << LEVEL       # 256
BLK = T // NODES         # 256 samples per block
THRESHOLD = 0.1
RPB = 2                  # rows per batch (inner tile loop)
NBATCH = ROWS // RPB     # 8 batches per core

# fp32r matmuls: 4x faster PE, ~1e-4 relative error.  Set False for exact fp32.
USE_F32R = True
TRANSPOSE_F32R = True    # also run the PE transposes in f32r (1.5 vs 2 cyc/row)

_INV = np.float32(0.7071067811865476)
_g = np.arange(NODES)
_FREQ_PERM = np.argsort(_g ^ (_g >> 1))


def _build_w():
    """W'[n, j]: response of freq-ordered node n to impulse at in-block pos j."""
    c = np.eye(BLK, dtype=np.float32)[:, None, :]
    for _ in range(LEVEL):
        ev = c[..., 0::2]
        od = c[..., 1::2]
        a = (ev + od) * _INV
        d = (ev - od) * _INV
        c = np.concatenate([a[:, :, None, :], d[:, :, None, :]], axis=2)
        c = c.reshape(BLK, -1, a.shape[-1])
    w = c[:, _FREQ_PERM, 0].T.copy()  # [n, j]
    return w


W = _build_w()
# lhsT chunks: wt[j', jc*256 + nh*128 + m] = W[nh*128+m, jc*128+j']
WT_PACKED = np.hstack([W.T[0:128, :], W.T[128:256, :]]).astype(np.float32).copy()
IDENT = np.eye(128, dtype=np.float32)

_MODULE_CACHE = None


def _build_module():
    nc = bacc.Bacc("TRN2", target_bir_lowering=False, debug=False,
                   enable_asserts=False, num_devices=N_CORES)
    FMM = F32R if USE_F32R else F32
    FX = F32R if (USE_F32R and TRANSPOSE_F32R) else F32
    x_d = nc.dram_tensor("x", [ROWS, T], FX, kind="ExternalInput").ap()
    wt_d = nc.dram_tensor("wt", [128, 512], FMM, kind="ExternalInput").ap()
    id_d = nc.dram_tensor("ident", [128, 128], FX, kind="ExternalInput").ap()
    id2_d = nc.dram_tensor("ident2", [128, 128], F32, kind="ExternalInput").ap()
    out_d = nc.dram_tensor("out", [ROWS, T], F32, kind="ExternalOutput").ap()
    ent_d = nc.dram_tensor("ent", [ROWS, NODES], F32, kind="ExternalOutput").ap()

    FREE = RPB * 512     # free-dim elems per batch tile (2 rows x 512)

    with tile.TileContext(nc) as tc:
        with (
            tc.tile_pool(name="const", bufs=1) as const_pool,
            tc.tile_pool(name="xin", bufs=3) as xin_pool,
            tc.tile_pool(name="xt_ps", bufs=2, space="PSUM") as xtps_pool,
            tc.tile_pool(name="c_ps", bufs=3, space="PSUM") as cps_pool,
            tc.tile_pool(name="xt_sb", bufs=2) as xtsb_pool,
            tc.tile_pool(name="sq", bufs=2) as sq_pool,
            tc.tile_pool(name="lq", bufs=2) as lq_pool,
            tc.tile_pool(name="tt", bufs=2) as tt_pool,
            tc.tile_pool(name="outs", bufs=3) as out_pool,
            tc.tile_pool(name="stats", bufs=3) as stat_pool,
        ):
            wt_sb = const_pool.tile([128, 512], FMM)
            nc.sync.dma_start(wt_sb[:], wt_d)
            ident = const_pool.tile([128, 128], FX)
            nc.sync.dma_start(ident[:], id_d)
            ident2 = const_pool.tile([128, 128], F32)
            nc.sync.dma_start(ident2[:], id2_d)
            ent_sb = const_pool.tile([128, 2 * ROWS], F32)
            bias_tiny = const_pool.tile([128, 1], F32)
            nc.gpsimd.memset(bias_tiny[:], 1e-30)

            for bi in range(NBATCH):
                r0 = bi * RPB
                x_sb = xin_pool.tile([128, FREE], FX)
                src = x_d[r0:r0 + RPB].rearrange(
                    "r (cj p j) -> p r cj j", cj=2, p=128, j=BLK)
                nc.sync.dma_start(x_sb[:], src)

                # PE transposes, per row rl into its own 1-bank PSUM tile:
                #   xt_r[:, (jc*2+cj)*128 + k'] , partitions j'
                xts = xtsb_pool.tile([128, FREE], FMM)
                for rl in range(RPB):
                    xt_r = xtps_pool.tile([128, 512], FX, tag="xt")
                    for cj in range(2):
                        for jc in range(2):
                            nc.tensor.transpose(
                                xt_r[:, (jc * 2 + cj) * 128:
                                     (jc * 2 + cj + 1) * 128],
                                x_sb[:, rl * 512 + cj * 256 + jc * 128:
                                     rl * 512 + cj * 256 + (jc + 1) * 128],
                                ident[:],
                            )
                    # PSUM->SBUF (+ f32r rounding); alternate engines
                    if rl % 2 == 0:
                        nc.vector.tensor_copy(
                            xts[:, rl * 512:(rl + 1) * 512], xt_r[:])
                    else:
                        nc.scalar.copy(
                            xts[:, rl * 512:(rl + 1) * 512], xt_r[:])

                # matmuls, N=512 moving (both rows per instruction):
                #   c[:, nh*512 + rl*256 + k] = coeffs[nh*128+n', k] of row rl
                c = cps_pool.tile([128, FREE], F32)
                xts_r = xts[:].rearrange("p (rl jc ck) -> p jc rl ck",
                                         rl=RPB, jc=2, ck=256)
                for nh in range(2):
                    for jc in range(2):
                        nc.tensor.matmul(
                            c[:, nh * 512:(nh + 1) * 512],
                            wt_sb[:, jc * 256 + nh * 128:
                                  jc * 256 + (nh + 1) * 128],
                            xts_r[:, jc],
                            start=(jc == 0), stop=(jc == 1),
                        )

                # entropy pipeline (segment order now (nh, rl))
                sq = sq_pool.tile([128, FREE], F32)
                nc.scalar.square(sq[:], c[:])
                s4 = stat_pool.tile([128, 2 * RPB], F32, tag="s4")
                nc.vector.tensor_reduce(
                    s4[:], sq[:].rearrange("p (g k) -> p g k", k=BLK),
                    axis=mybir.AxisListType.X, op=ALU.add)
                lq = lq_pool.tile([128, FREE], F32)
                nc.scalar.activation(lq[:], sq[:], ACTFN.Ln, bias=bias_tiny[:])
                t = tt_pool.tile([128, FREE], F32)
                nc.gpsimd.tensor_tensor(t[:], sq[:], lq[:], op=ALU.mult)
                g4 = stat_pool.tile([128, 2 * RPB], F32, tag="g4")
                nc.vector.tensor_reduce(
                    g4[:], t[:].rearrange("p (g k) -> p g k", k=BLK),
                    axis=mybir.AxisListType.X, op=ALU.add)

                se = stat_pool.tile([128, 2 * RPB], F32, tag="se")
                nc.vector.tensor_scalar_add(se[:], s4[:], 1e-8)
                sinv = stat_pool.tile([128, 2 * RPB], F32, tag="sinv")
                nc.vector.reciprocal(sinv[:], se[:])
                lns = stat_pool.tile([128, 2 * RPB], F32, tag="lns")
                nc.scalar.activation(lns[:], se[:], ACTFN.Ln)
                u = stat_pool.tile([128, 2 * RPB], F32, tag="u")
                nc.vector.tensor_tensor(u[:], s4[:], lns[:], op=ALU.mult)
                v = stat_pool.tile([128, 2 * RPB], F32, tag="v")
                nc.vector.tensor_tensor(v[:], u[:], g4[:], op=ALU.subtract)
                # e columns are (nh, rl); ent_sb wants col 2*(r0+rl)+nh
                e = ent_sb[:, 2 * r0: 2 * (r0 + RPB)].rearrange(
                    "p (rl nh) -> p nh rl", rl=RPB, nh=2)
                nc.vector.tensor_tensor(e, v[:], sinv[:], op=ALU.mult)
                m4 = stat_pool.tile([128, 2 * RPB], F32, tag="m4")
                nc.vector.tensor_scalar(
                    m4[:], ent_sb[:, 2 * r0: 2 * (r0 + RPB)].rearrange(
                        "p (rl nh) -> p nh rl", rl=RPB, nh=2),
                    THRESHOLD, None, op0=ALU.is_gt)

                # masked output; out layout (rl, nh, k) for the DMA
                o = out_pool.tile([128, FREE], F32)
                for rl in range(RPB):
                    for nh in range(2):
                        csl = c[:, nh * 512 + rl * 256: nh * 512 + (rl + 1) * 256]
                        osl = o[:, rl * 512 + nh * 256: rl * 512 + (nh + 1) * 256]
                        msl = m4[:, nh * RPB + rl: nh * RPB + rl + 1]
                        if (rl * 2 + nh) % 2 == 0:
                            nc.vector.tensor_scalar_mul(osl, csl, msl)
                        else:
                            nc.scalar.activation(osl, csl, ACTFN.Copy, scale=msl)
                dst = out_d[r0:r0 + RPB].rearrange(
                    "r (nh p k) -> p r nh k", nh=2, p=128, k=BLK)
                nc.sync.dma_start(dst, o[:])

            # entropy epilogue: [128 n', 32 (r,nh)] -> transpose -> [32, 128] -> DRAM
            entT_ps = xtps_pool.tile([128, 512], F32, tag="xt")
            nc.tensor.transpose(entT_ps[0:2 * ROWS, 0:128], ent_sb[:], ident2[:])
            entT = stat_pool.tile([2 * ROWS, 128], F32, tag="entT_sb")
            nc.vector.tensor_copy(entT[:], entT_ps[0:2 * ROWS, 0:128])
            nc.sync.dma_start(
                ent_d.rearrange("r (nh n) -> (r nh) n", nh=2), entT[:])

    nc.compile()
    return nc


def _get_module():
    global _MODULE_CACHE
    if _MODULE_CACHE is None:
        _MODULE_CACHE = _build_module()
    return _MODULE_CACHE


def kernel(x: np.ndarray) -> tuple[np.ndarray, np.ndarray, np.ndarray]:
    x = np.ascontiguousarray(np.asarray(x, dtype=np.float32))
    assert x.shape == (B, T)
    nc = _get_module()
    in_maps = []
    for core in range(N_CORES):
        shard = x[core * ROWS:(core + 1) * ROWS]
        in_maps.append({"x": shard, "wt": WT_PACKED, "ident": IDENT,
                        "ident2": IDENT})
    res = bass_utils.run_bass_kernel_spmd(nc, in_maps, core_ids=list(range(N_CORES)))
    global LAST_RESULTS
    LAST_RESULTS = res
    out = np.empty((B, NODES, BLK), dtype=np.float32)
    ent = np.empty((B, NODES), dtype=np.float32)
    for core in range(N_CORES):
        r = res.results[core]
        out[core * ROWS:(core + 1) * ROWS] = r["out"].reshape(ROWS, NODES, BLK)
        ent[core * ROWS:(core + 1) * ROWS] = r["ent"]
    keep = ent > np.float32(THRESHOLD)
    return out, ent, keep
